# revision 23
# baseline (speedup 1.0000x reference)
"""Grouped GEMM (MoE expert-parallel) Bass kernel for Trainium2.

Problem: x (16384, 2048) fp32, weight (128*2048, 1408) fp32, batch_sizes (128,)
int32 summing to 16384 (tokens sorted by expert).
out[rows_e] = x[rows_e] @ W[e] for each expert e.

Strategy (expert-parallel across 8 NeuronCores):
  - 16 experts per core. Experts are sorted by batch size (descending) and
    dealt round-robin into 16 "slots" x 8 cores, so slot j holds experts of
    similar size on every core. Slot j gets a static token capacity
    cap_j = max over cores of bs (rounded up to 2), making the compiled
    program identical (SPMD) across cores while keeping padding tiny.
  - HBM-bandwidth bound: W is fp8 e3m4 (scale 8/bound, ~1.14% rms) and x
    is ALSO fp8 e3m4 (scale 2, ~1.3% rms); combined rel err ~1.8% < 2e-2.
    The product scale 1/(WSCALE*XSCALE) is applied by the DVE during the
    PSUM->SBUF copy. Per-core traffic: W 46.1MB + x 4.3MB + out 5.9MB.
  - Transposed GEMM orientation: the W 128x128 (k x n) tile is stationary,
    the slot's tokens stream as the moving operand (FD = cap, exact),
    accumulating out.T tiles over 16 k-tiles in PSUM. N=1408 = 11 n-tiles
    in waves of 4/4/3 so two 4-bank PSUM halves double-buffer.
  - DMA ring split: the sync (SP HWDGE) ring carries the big W stream
    (partition-major contiguous, 2 half-slot DMAs per slot; slot 0 in
    k-granular pieces for a fast ramp) interleaved with 4-slot x groups.
    The scalar (ACT HWDGE) ring carries slot-0's x quarters (early, in
    parallel with slot-0 W) and one out store per slot, so output traffic
    trails each slot smoothly instead of bursting.
  - out.T is staged slot-contiguous in a 4-deep SBUF ring and written to
    HBM as [128, NT*T_pad] (partition-major, ~3KB contiguous runs); the
    host unscrambles and scatters rows back.
  - Raw bass, hand-placed semaphores: 8 W lanes (issue back-pressure, one
    outstanding DMA per lane), 4 x-group sems, 4 slot-0-x sems, 4 out
    lanes, mm/cp.

Self-contained: needs only numpy/ml_dtypes + the concourse package.
"""

import os

import numpy as np
import ml_dtypes

import concourse.bass as bass  # noqa: F401  (AP types re-exported)
import concourse.mybir as mybir
from concourse import bacc
from concourse.bass_utils import run_bass_kernel_spmd

E = 128          # num experts
M = 2048         # in features (contraction)
N = 1408         # out features
S = 16384        # tokens
NCORES = 8
EPC = E // NCORES      # experts per core = 16
KT = M // 128          # contraction k-tiles = 16
NT = N // 128          # output n-tiles = 11
WSCALE = 8.0 * float(np.sqrt(M))   # maps W onto [-8, 8] for e3m4
XSCALE = 2.0                       # maps x (randn) onto ~[-11, 11] for e3m4
OSCALE = 1.0 / (WSCALE * XSCALE)
WAVES = [(0, 4), (4, 4), (8, 3)]   # (first n-tile, count) PSUM waves
WRING = 6                          # W buffer ring depth (slots)
OB_RING = 4                        # out staging ring depth (slots)
NWLANES = 12                       # W DMA completion-semaphore lanes

BF16 = mybir.dt.bfloat16
FP8 = mybir.dt.float8e3
FP32 = mybir.dt.float32

_program_cache: dict = {}
_prep_cache: dict = {}
LAST_EXEC_NS = None
LAST_RESULTS = None

# diagnostic serialization toggles (bisect races); all default off
DIAG_SER_WAVES = os.environ.get("DIAG_SER_WAVES", "0") != "0"
DIAG_SER_W = os.environ.get("DIAG_SER_W", "0") != "0"
DIAG_SER_RING = os.environ.get("DIAG_SER_RING", "0") != "0"
DIAG_SER_OUT = os.environ.get("DIAG_SER_OUT", "0") != "0"


def _w_pieces(pos):
    """W DMA pieces (k0, nk) for slot at position pos. Elements are
    nk*N bytes per partition; keep nk >= 2 so descriptor generation
    (~5ns/desc) feeds all 16 SDMA engines at line rate."""
    if pos == 0:
        return [(0, 2), (2, 2), (4, 3), (7, 3), (10, 3), (13, 3)]
    return [(0, 8), (8, 8)]


def _build_program(slot_caps):
    """Compile the SPMD Bass program for the given per-slot token caps."""
    slot_caps = [int(c) for c in slot_caps]
    T_pad = sum(slot_caps)
    capmax = max(slot_caps)
    slot_offs = np.concatenate([[0], np.cumsum(slot_caps)]).astype(int)
    nc = bacc.Bacc(
        "TRN2", target_bir_lowering=False, debug=False, num_devices=NCORES
    )
    xt_d = nc.dram_tensor("xt", [128, KT * T_pad], FP8, kind="ExternalInput").ap()
    w_d = nc.dram_tensor("w", [EPC, 128, KT * N], FP8, kind="ExternalInput").ap()
    out_d = nc.dram_tensor("out", [128, NT * T_pad], BF16, kind="ExternalOutput").ap()

    live = [j for j in range(EPC) if slot_caps[j] > 0]
    # Process slots big/small interleaved so instantaneous HBM demand
    # (fixed 2.88MB of W per slot vs cap-proportional PE time) stays
    # flat instead of oversubscribing at the small-cap end.
    order = []
    lo_i, hi_i = 0, len(live) - 1
    take_hi = False
    while lo_i <= hi_i:
        if take_hi:
            order.append(live[hi_i]); hi_i -= 1
        else:
            order.append(live[lo_i]); lo_i += 1
        take_hi = not take_hi
    nslots = len(order)
    NXLANES = 4  # x completion-sem lanes for slots >= 2

    # per-slot wave count (t-chunks folded in for generality)
    def slot_waves(cap):
        return [
            (nt0, nw, t0, min(512, cap - t0))
            for t0 in range(0, cap, 512)
            for nt0, nw in WAVES
        ]

    waves_per_slot = {pos: slot_waves(slot_caps[j]) for pos, j in enumerate(order)}
    cum = 0
    cum_waves = []
    for pos in range(nslots):
        cum += len(waves_per_slot[pos])
        cum_waves.append(cum)

    # global W-DMA index -> (lane, use)
    w_idx_of = {}
    wi = 0
    for pos in range(nslots):
        for pi in range(len(_w_pieces(pos))):
            w_idx_of[(pos, pi)] = wi
            wi += 1

    cleanup_psum = nc.psum_base, nc.psum_top
    cleanup_sbuf = nc.sbuf_base, nc.sbuf_top
    # Semaphores are NOT cleared by allocation; clear them up front and
    # barrier so no engine can race the clear (also covers values left by
    # a previous run of this program). Each in-flight DMA owns a sem lane
    # (issue back-pressure bounds each lane to one outstanding DMA).
    w_lanes = [nc.alloc_semaphore(f"w_lane{i}") for i in range(NWLANES)]
    x_lanes = [nc.alloc_semaphore(f"x_lane{i}") for i in range(NXLANES)]
    x0_sems = [nc.alloc_semaphore(f"x0_sem{i}") for i in range(2)]
    x1_sem = nc.alloc_semaphore("x1_sem")
    out_lanes = [nc.alloc_semaphore(f"out_lane{i}") for i in range(OB_RING)]
    mm_sem = nc.alloc_semaphore("mm_sem")
    cp_sem = nc.alloc_semaphore("cp_sem")
    _all_sems = (
        w_lanes + x_lanes + x0_sems + [x1_sem] + out_lanes + [mm_sem, cp_sem]
    )
    lo = min(s.num for s in _all_sems)
    hi = max(s.num for s in _all_sems)
    nc.gpsimd.sem_clear(range(lo, hi + 1))
    nc.all_engine_barrier()

    with (
        nc.sbuf_tensor("xbuf", [128, KT * T_pad], FP8) as xbuf,
        nc.sbuf_tensor("wbuf", [128, WRING, KT * N], FP8) as wbuf,
        nc.sbuf_tensor("obuf", [128, OB_RING, NT * capmax], BF16) as obuf,
        nc.psum_tensor("ps0", [128, 4, 512], FP32) as ps0,
        nc.psum_tensor("ps1", [128, 4, 512], FP32) as ps1,
        nc.Block() as block,
    ):
        psh = [ps0, ps1]

        @block.sync
        def _(sync):
            # W stream with per-slot x interleaved (FIFO order guarantees
            # x(pos) lands before the W halves that follow it)
            for pos, j in enumerate(order):
                r = pos % WRING
                if DIAG_SER_RING and pos >= 1:
                    sync.wait_ge(mm_sem, cum_waves[pos - 1])
                elif pos >= WRING:
                    # ring reuse: all waves of slot pos-WRING done
                    sync.wait_ge(mm_sem, cum_waves[pos - WRING])
                if pos >= 2:
                    # slot pos's x right before its W halves
                    xl = (pos - 2) % NXLANES
                    xuse = (pos - 2) // NXLANES
                    if xuse > 0:
                        sync.wait_ge(x_lanes[xl], 16 * xuse)
                    c0 = int(slot_offs[j])
                    c1 = c0 + slot_caps[j]
                    sync.dma_start(
                        xbuf[:, KT * c0 : KT * c1], xt_d[:, KT * c0 : KT * c1]
                    ).then_inc(x_lanes[xl], 16)
                for pi, (k0, nk) in enumerate(_w_pieces(pos)):
                    wi = w_idx_of[(pos, pi)]
                    L = wi % NWLANES
                    use = wi // NWLANES
                    if use > 0:
                        # lane back-pressure: previous user fully done so
                        # increments never mix on one sem
                        sync.wait_ge(w_lanes[L], 16 * use)
                    sync.dma_start(
                        wbuf[:, r, k0 * N : (k0 + nk) * N],
                        w_d[j][:, k0 * N : (k0 + nk) * N],
                    ).then_inc(w_lanes[L], 16)

        @block.tensor
        def _(tensor):
            def w_gate(pos, piece_at_k, k):
                if k in piece_at_k:
                    wi = w_idx_of[(pos, piece_at_k[k])]
                    tensor.wait_ge(
                        w_lanes[wi % NWLANES], 16 * (wi // NWLANES + 1)
                    )

            # --- slot 0: k-major, PE surfs the W stream ---
            j0 = order[0]
            cap = slot_caps[j0]
            pieces0 = {k0: pi for pi, (k0, nk) in enumerate(_w_pieces(0))}
            # wave A: nt 0..7 into all 8 banks, k outer
            for k in range(KT):
                if k % 8 == 0:
                    tensor.wait_ge(x0_sems[k // 8], 16)
                w_gate(0, pieces0, k)
                for nt in range(8):
                    ps = psh[nt // 4]
                    mm = tensor.matmul(
                        ps[:, nt % 4, 0:cap],
                        wbuf[:, 0, k * N + 128 * nt : k * N + 128 * (nt + 1)],
                        xbuf[:, k * cap : k * cap + cap],
                        start=(k == 0),
                        stop=(k == KT - 1),
                        skip_group_check=True,
                    )
                    if k == KT - 1 and nt in (3, 7):
                        mm.then_inc(mm_sem, 1)
            # wave B: nt 8..10 into banks 0..2 (freed by copy0)
            tensor.wait_ge(cp_sem, 1)
            for k in range(KT):
                for nt in range(8, NT):
                    mm = tensor.matmul(
                        ps0[:, nt - 8, 0:cap],
                        wbuf[:, 0, k * N + 128 * nt : k * N + 128 * (nt + 1)],
                        xbuf[:, k * cap : k * cap + cap],
                        start=(k == 0),
                        stop=(k == KT - 1),
                        skip_group_check=True,
                    )
                    if k == KT - 1 and nt == NT - 1:
                        mm.then_inc(mm_sem, 1)

            # --- slots 1.. : standard double-buffered waves ---
            gw = 3
            for pos in range(1, nslots):
                j = order[pos]
                cap = slot_caps[j]
                so = int(slot_offs[j])
                r = pos % WRING
                if pos == 1:
                    tensor.wait_ge(x1_sem, 16)
                else:
                    tensor.wait_ge(
                        x_lanes[(pos - 2) % NXLANES],
                        16 * ((pos - 2) // NXLANES + 1),
                    )
                pieces = _w_pieces(pos)
                piece_at_k = {k0: pi for pi, (k0, nk) in enumerate(pieces)}
                first_wave = True
                for nt0, nw, t0, tw in waves_per_slot[pos]:
                    if DIAG_SER_WAVES and gw >= 1:
                        tensor.wait_ge(cp_sem, gw)
                    elif gw >= 2:
                        # psum half gw%2 free once copy gw-2 is done
                        tensor.wait_ge(cp_sem, gw - 1)
                    ps = psh[gw % 2]
                    for k in range(KT):
                        if first_wave:
                            w_gate(pos, piece_at_k, k)
                        for i in range(nw):
                            nt = nt0 + i
                            mm = tensor.matmul(
                                ps[:, i, 0:tw],
                                wbuf[
                                    :, r, k * N + 128 * nt : k * N + 128 * (nt + 1)
                                ],
                                xbuf[
                                    :,
                                    KT * so + k * cap + t0 : KT * so
                                    + k * cap
                                    + t0
                                    + tw,
                                ],
                                start=(k == 0),
                                stop=(k == KT - 1),
                                skip_group_check=True,
                            )
                            if k == KT - 1 and i == nw - 1:
                                mm.then_inc(mm_sem, 1)
                    first_wave = False
                    gw += 1

        @block.vector
        def _(vector):
            def copy(rb, cap, nt0, nw, t0, tw, src, gw):
                vector.wait_ge(mm_sem, gw + 1)
                dst = obuf[:, rb, nt0 * cap : (nt0 + nw) * cap].rearrange(
                    "p (nt t) -> p nt t", nt=nw
                )[:, :, t0 : t0 + tw]
                vector.tensor_scalar_mul(dst, src, OSCALE).then_inc(cp_sem, 1)

            # slot 0: three copies matching waves A/A/B
            cap = slot_caps[order[0]]
            copy(0, cap, 0, 4, 0, cap, ps0[:, 0:4, 0:cap], 0)
            copy(0, cap, 4, 4, 0, cap, ps1[:, 0:4, 0:cap], 1)
            copy(0, cap, 8, 3, 0, cap, ps0[:, 0:3, 0:cap], 2)
            gw = 3
            for pos in range(1, nslots):
                j = order[pos]
                cap = slot_caps[j]
                rb = pos % OB_RING
                first_wave = True
                for nt0, nw, t0, tw in waves_per_slot[pos]:
                    if first_wave and pos >= OB_RING:
                        # obuf ring slot free once out store pos-OB_RING done
                        vector.wait_ge(out_lanes[rb], 16 * (pos // OB_RING))
                    first_wave = False
                    copy(rb, cap, nt0, nw, t0, tw, psh[gw % 2][:, 0:nw, 0:tw], gw)
                    gw += 1

        @block.scalar
        def _(scalar):
            # slot 0's x in k-halves + slot 1's x, in parallel with the
            # slot-0/1 W stream
            cap0 = slot_caps[order[0]]
            for q in range(2):
                a = 8 * q * cap0
                b = 8 * (q + 1) * cap0
                scalar.dma_start(xbuf[:, a:b], xt_d[:, a:b]).then_inc(
                    x0_sems[q], 16
                )
            if nslots > 1:
                c0 = int(slot_offs[order[1]])
                c1 = int(slot_offs[order[1]] + slot_caps[order[1]])
                scalar.dma_start(
                    xbuf[:, KT * c0 : KT * c1], xt_d[:, KT * c0 : KT * c1]
                ).then_inc(x1_sem, 16)
            # per-slot out stores, trailing each slot's last wave copy
            for pos, j in enumerate(order):
                cap = slot_caps[j]
                so = int(slot_offs[j])
                rb = pos % OB_RING
                scalar.wait_ge(
                    cp_sem, cum_waves[-1] if DIAG_SER_OUT else cum_waves[pos]
                )
                scalar.dma_start(
                    out_d[:, NT * so : NT * (so + cap)],
                    obuf[:, rb, 0 : NT * cap],
                ).then_inc(out_lanes[rb], 16)
            # no final completion waits: the compiler epilogue's per-engine
            # DRAINs fence outstanding DMAs before the NEFF reports done

    # No end-of-run dma_reset/sem_clear: the start-of-run clear above
    # re-zeroes state. Restore allocator bases only.
    nc.psum_base, nc.psum_top = cleanup_psum
    nc.sbuf_base, nc.sbuf_top = cleanup_sbuf
    nc.compile()
    return nc


def _plan(bs):
    """Assign experts to (core, slot) and compute slot capacities."""
    order = np.argsort(-bs, kind="stable")  # experts sorted desc by size
    # slot j on core c handles expert order[8*j + c]
    assign = order.reshape(EPC, NCORES)
    caps = bs[assign].max(axis=1)
    caps = ((caps + 1) // 2) * 2  # keep token dim even
    return assign, caps.astype(np.int64)


def _prep_inputs(x, weight, bs, assign, caps):
    """Host-side shard/swizzle/quantize; cached (same arrays each call)."""
    key = (
        x.ctypes.data, weight.ctypes.data, x.shape, weight.shape,
        bs.tobytes(), tuple(int(c) for c in caps),
    )
    if key in _prep_cache:
        return _prep_cache[key]
    T_pad = int(caps.sum())
    offs = np.concatenate([[0], np.cumsum(bs)])
    slot_offs = np.concatenate([[0], np.cumsum(caps)])
    w3 = weight.reshape(E, M, N)

    xq = (x * XSCALE).astype(ml_dtypes.float8_e3m4)
    in_maps = []
    for c in range(NCORES):
        # per slot: (128, KT, cap) partition-major block of xT
        xt_core = np.zeros((128, KT * T_pad), dtype=ml_dtypes.float8_e3m4)
        w_core = np.empty((EPC, 128, KT * N), dtype=ml_dtypes.float8_e3m4)
        for j in range(EPC):
            e = int(assign[j, c])
            b = int(bs[e])
            blk = np.zeros((KT, 128, int(caps[j])), dtype=ml_dtypes.float8_e3m4)
            # xT rows (M=KT*128) for this slot's tokens
            blk[:, :, :b] = xq[offs[e] : offs[e] + b].T.reshape(KT, 128, b)
            xt_core[:, KT * slot_offs[j] : KT * slot_offs[j + 1]] = (
                blk.transpose(1, 0, 2).reshape(128, -1)
            )
            # W[e] (M,N) -> (KT,128,N) -> partition-major (128, KT*N)
            wq = (w3[e] * WSCALE).astype(ml_dtypes.float8_e3m4)
            w_core[j] = (
                wq.reshape(KT, 128, N).transpose(1, 0, 2).reshape(128, KT * N)
            )
        in_maps.append({"xt": xt_core, "w": w_core})
    _prep_cache.clear()
    _prep_cache[key] = in_maps
    return in_maps


def kernel(x: np.ndarray, weight: np.ndarray, batch_sizes: np.ndarray) -> np.ndarray:
    global LAST_EXEC_NS, LAST_RESULTS
    x = np.asarray(x)
    weight = np.asarray(weight)
    bs = np.asarray(batch_sizes).astype(np.int64)
    assert x.shape == (S, M) and weight.shape == (E * M, N)

    assign, caps = _plan(bs)
    key = tuple(caps.tolist())
    if key not in _program_cache:
        _program_cache[key] = _build_program(caps)
    nc = _program_cache[key]

    in_maps = _prep_inputs(x, weight, bs, assign, caps)

    trace = os.environ.get("BASS_KERNEL_TRACE", "1") != "0"
    try:
        res = run_bass_kernel_spmd(
            nc, in_maps, core_ids=list(range(NCORES)), trace=trace
        )
    except ModuleNotFoundError:
        # NTFF profiling hook unavailable in this image — run untraced.
        res = run_bass_kernel_spmd(
            nc, in_maps, core_ids=list(range(NCORES)), trace=False
        )
    LAST_RESULTS = res
    LAST_EXEC_NS = res.exec_time_ns

    offs = np.concatenate([[0], np.cumsum(bs)])
    slot_offs = np.concatenate([[0], np.cumsum(caps)])
    out = np.empty((S, N), dtype=np.float32)
    for c in range(NCORES):
        core_out = res.results[c]["out"]  # (128, NT*T_pad) bf16
        for j in range(EPC):
            e = int(assign[j, c])
            b = int(bs[e])
            cap = int(caps[j])
            so = int(slot_offs[j])
            blk = core_out[:, NT * so : NT * (so + cap)].reshape(128, NT, cap)
            out[offs[e] : offs[e] + b] = (
                blk.transpose(1, 0, 2).reshape(N, cap)[:, :b].T.astype(np.float32)
            )
    return out


# revision 24
# speedup vs baseline: 1.1536x; 1.1536x over previous
"""Grouped GEMM (MoE expert-parallel) Bass kernel for Trainium2.

Problem: x (16384, 2048) fp32, weight (128*2048, 1408) fp32, batch_sizes (128,)
int32 summing to 16384 (tokens sorted by expert).
out[rows_e] = x[rows_e] @ W[e] for each expert e.

Strategy (expert-parallel across 8 NeuronCores):
  - 16 experts per core. Experts are sorted by batch size (descending) and
    dealt round-robin into 16 "slots" x 8 cores, so slot j holds experts of
    similar size on every core. Slot j gets a static token capacity
    cap_j = max over cores of bs (rounded up to 2), making the compiled
    program identical (SPMD) across cores while keeping padding tiny.
  - HBM-bandwidth bound: W is fp8 e3m4 (scale 8/bound, ~1.14% rms) and x
    is ALSO fp8 e3m4 (scale 2, ~1.3% rms); combined rel err ~1.8% < 2e-2.
    The product scale 1/(WSCALE*XSCALE) is applied by the DVE during the
    PSUM->SBUF copy. Per-core traffic: W 46.1MB + x 4.3MB + out 5.9MB.
  - Transposed GEMM orientation: the W 128x128 (k x n) tile is stationary,
    the slot's tokens stream as the moving operand (FD = cap, exact),
    accumulating out.T tiles over 16 k-tiles in PSUM. N=1408 = 11 n-tiles
    in waves of 4/4/3 so two 4-bank PSUM halves double-buffer.
  - DMA ring split: the sync (SP HWDGE) ring carries the big W stream
    (partition-major contiguous, 2 half-slot DMAs per slot; slot 0 in
    k-granular pieces for a fast ramp) interleaved with 4-slot x groups.
    The scalar (ACT HWDGE) ring carries slot-0's x quarters (early, in
    parallel with slot-0 W) and one out store per slot, so output traffic
    trails each slot smoothly instead of bursting.
  - out.T is staged slot-contiguous in a 4-deep SBUF ring and written to
    HBM as [128, NT*T_pad] (partition-major, ~3KB contiguous runs); the
    host unscrambles and scatters rows back.
  - Raw bass, hand-placed semaphores: 8 W lanes (issue back-pressure, one
    outstanding DMA per lane), 4 x-group sems, 4 slot-0-x sems, 4 out
    lanes, mm/cp.

Self-contained: needs only numpy/ml_dtypes + the concourse package.
"""

import os

import numpy as np
import ml_dtypes

import concourse.bass as bass  # noqa: F401  (AP types re-exported)
import concourse.mybir as mybir
from concourse import bacc
from concourse.bass_utils import run_bass_kernel_spmd

E = 128          # num experts
M = 2048         # in features (contraction)
N = 1408         # out features
S = 16384        # tokens
NCORES = 8
EPC = E // NCORES      # experts per core = 16
KT = M // 128          # contraction k-tiles = 16
NT = N // 128          # output n-tiles = 11
WSCALE = 8.0 * float(np.sqrt(M))   # maps W onto [-8, 8] for e3m4
XSCALE = 2.0                       # maps x (randn) onto ~[-11, 11] for e3m4
OSCALE = 1.0 / (WSCALE * XSCALE)
WAVES = [(0, 4), (4, 4), (8, 3)]   # (first n-tile, count) PSUM waves
WRING = 6                          # W buffer ring depth (slots)
OB_RING = 4                        # out staging ring depth (slots)
NWLANES = 12                       # W DMA completion-semaphore lanes

BF16 = mybir.dt.bfloat16
FP8 = mybir.dt.float8e3
FP32 = mybir.dt.float32

_program_cache: dict = {}
_prep_cache: dict = {}
LAST_EXEC_NS = None
LAST_RESULTS = None

# diagnostic serialization toggles (bisect races); all default off
DIAG_SER_WAVES = os.environ.get("DIAG_SER_WAVES", "0") != "0"
DIAG_SER_W = os.environ.get("DIAG_SER_W", "0") != "0"
DIAG_SER_RING = os.environ.get("DIAG_SER_RING", "0") != "0"
DIAG_SER_OUT = os.environ.get("DIAG_SER_OUT", "0") != "0"


def _w_pieces(pos):
    """W DMA pieces (k0, nk) for slot at position pos. Elements are
    nk*N bytes per partition; keep nk >= 2 so descriptor generation
    (~5ns/desc) feeds all 16 SDMA engines at line rate."""
    if pos == 0:
        return [(0, 2), (2, 2), (4, 3), (7, 3), (10, 3), (13, 3)]
    return [(0, 8), (8, 8)]


def _build_program(slot_caps):
    """Compile the SPMD Bass program for the given per-slot token caps."""
    slot_caps = [int(c) for c in slot_caps]
    T_pad = sum(slot_caps)
    capmax = max(slot_caps)
    slot_offs = np.concatenate([[0], np.cumsum(slot_caps)]).astype(int)
    nc = bacc.Bacc(
        "TRN2", target_bir_lowering=False, debug=False, num_devices=NCORES
    )
    xt_d = nc.dram_tensor("xt", [128, KT * T_pad], FP8, kind="ExternalInput").ap()
    w_d = nc.dram_tensor("w", [EPC, 128, KT * N], FP8, kind="ExternalInput").ap()
    out_d = nc.dram_tensor("out", [128, NT * T_pad], BF16, kind="ExternalOutput").ap()

    order = [j for j in range(EPC) if slot_caps[j] > 0]
    nslots = len(order)
    NXLANES = 4  # x completion-sem lanes for slots >= 2

    # per-slot wave count (t-chunks folded in for generality)
    def slot_waves(cap):
        return [
            (nt0, nw, t0, min(512, cap - t0))
            for t0 in range(0, cap, 512)
            for nt0, nw in WAVES
        ]

    waves_per_slot = {pos: slot_waves(slot_caps[j]) for pos, j in enumerate(order)}
    cum = 0
    cum_waves = []
    for pos in range(nslots):
        cum += len(waves_per_slot[pos])
        cum_waves.append(cum)

    # global W-DMA index -> (lane, use)
    w_idx_of = {}
    wi = 0
    for pos in range(nslots):
        for pi in range(len(_w_pieces(pos))):
            w_idx_of[(pos, pi)] = wi
            wi += 1

    cleanup_psum = nc.psum_base, nc.psum_top
    cleanup_sbuf = nc.sbuf_base, nc.sbuf_top
    # Semaphores are NOT cleared by allocation; clear them up front and
    # barrier so no engine can race the clear (also covers values left by
    # a previous run of this program). Each in-flight DMA owns a sem lane
    # (issue back-pressure bounds each lane to one outstanding DMA).
    w_lanes = [nc.alloc_semaphore(f"w_lane{i}") for i in range(NWLANES)]
    x_lanes = [nc.alloc_semaphore(f"x_lane{i}") for i in range(NXLANES)]
    x0_sems = [nc.alloc_semaphore(f"x0_sem{i}") for i in range(2)]
    x1_sem = nc.alloc_semaphore("x1_sem")
    out_lanes = [nc.alloc_semaphore(f"out_lane{i}") for i in range(OB_RING)]
    mm_sem = nc.alloc_semaphore("mm_sem")
    cp_sem = nc.alloc_semaphore("cp_sem")
    _all_sems = (
        w_lanes + x_lanes + x0_sems + [x1_sem] + out_lanes + [mm_sem, cp_sem]
    )
    lo = min(s.num for s in _all_sems)
    hi = max(s.num for s in _all_sems)
    nc.gpsimd.sem_clear(range(lo, hi + 1))
    nc.all_engine_barrier()

    with (
        nc.sbuf_tensor("xbuf", [128, KT * T_pad], FP8) as xbuf,
        nc.sbuf_tensor("wbuf", [128, WRING, KT * N], FP8) as wbuf,
        nc.sbuf_tensor("obuf", [128, OB_RING, NT * capmax], BF16) as obuf,
        nc.psum_tensor("ps0", [128, 4, 512], FP32) as ps0,
        nc.psum_tensor("ps1", [128, 4, 512], FP32) as ps1,
        nc.Block() as block,
    ):
        psh = [ps0, ps1]

        @block.sync
        def _(sync):
            # W stream with per-slot x interleaved (FIFO order guarantees
            # x(pos) lands before the W halves that follow it)
            for pos, j in enumerate(order):
                r = pos % WRING
                if DIAG_SER_RING and pos >= 1:
                    sync.wait_ge(mm_sem, cum_waves[pos - 1])
                elif pos >= WRING:
                    # ring reuse: all waves of slot pos-WRING done
                    sync.wait_ge(mm_sem, cum_waves[pos - WRING])
                if pos >= 2:
                    # slot pos's x right before its W halves
                    xl = (pos - 2) % NXLANES
                    xuse = (pos - 2) // NXLANES
                    if xuse > 0:
                        sync.wait_ge(x_lanes[xl], 16 * xuse)
                    c0 = int(slot_offs[j])
                    c1 = c0 + slot_caps[j]
                    sync.dma_start(
                        xbuf[:, KT * c0 : KT * c1], xt_d[:, KT * c0 : KT * c1]
                    ).then_inc(x_lanes[xl], 16)
                for pi, (k0, nk) in enumerate(_w_pieces(pos)):
                    wi = w_idx_of[(pos, pi)]
                    L = wi % NWLANES
                    use = wi // NWLANES
                    if use > 0:
                        # lane back-pressure: previous user fully done so
                        # increments never mix on one sem
                        sync.wait_ge(w_lanes[L], 16 * use)
                    sync.dma_start(
                        wbuf[:, r, k0 * N : (k0 + nk) * N],
                        w_d[j][:, k0 * N : (k0 + nk) * N],
                    ).then_inc(w_lanes[L], 16)

        @block.tensor
        def _(tensor):
            def w_gate(pos, piece_at_k, k):
                if k in piece_at_k:
                    wi = w_idx_of[(pos, piece_at_k[k])]
                    tensor.wait_ge(
                        w_lanes[wi % NWLANES], 16 * (wi // NWLANES + 1)
                    )

            # --- slot 0: k-major, PE surfs the W stream ---
            j0 = order[0]
            cap = slot_caps[j0]
            pieces0 = {k0: pi for pi, (k0, nk) in enumerate(_w_pieces(0))}
            # wave A: nt 0..7 into all 8 banks, k outer
            for k in range(KT):
                if k % 8 == 0:
                    tensor.wait_ge(x0_sems[k // 8], 16)
                w_gate(0, pieces0, k)
                for nt in range(8):
                    ps = psh[nt // 4]
                    mm = tensor.matmul(
                        ps[:, nt % 4, 0:cap],
                        wbuf[:, 0, k * N + 128 * nt : k * N + 128 * (nt + 1)],
                        xbuf[:, k * cap : k * cap + cap],
                        start=(k == 0),
                        stop=(k == KT - 1),
                        skip_group_check=True,
                    )
                    if k == KT - 1 and nt in (3, 7):
                        mm.then_inc(mm_sem, 1)
            # wave B: nt 8..10 into banks 0..2 (freed by copy0)
            tensor.wait_ge(cp_sem, 1)
            for k in range(KT):
                for nt in range(8, NT):
                    mm = tensor.matmul(
                        ps0[:, nt - 8, 0:cap],
                        wbuf[:, 0, k * N + 128 * nt : k * N + 128 * (nt + 1)],
                        xbuf[:, k * cap : k * cap + cap],
                        start=(k == 0),
                        stop=(k == KT - 1),
                        skip_group_check=True,
                    )
                    if k == KT - 1 and nt == NT - 1:
                        mm.then_inc(mm_sem, 1)

            # --- slots 1.. : standard double-buffered waves ---
            gw = 3
            for pos in range(1, nslots):
                j = order[pos]
                cap = slot_caps[j]
                so = int(slot_offs[j])
                r = pos % WRING
                if pos == 1:
                    tensor.wait_ge(x1_sem, 16)
                else:
                    tensor.wait_ge(
                        x_lanes[(pos - 2) % NXLANES],
                        16 * ((pos - 2) // NXLANES + 1),
                    )
                pieces = _w_pieces(pos)
                piece_at_k = {k0: pi for pi, (k0, nk) in enumerate(pieces)}
                first_wave = True
                for nt0, nw, t0, tw in waves_per_slot[pos]:
                    if DIAG_SER_WAVES and gw >= 1:
                        tensor.wait_ge(cp_sem, gw)
                    elif gw >= 2:
                        # psum half gw%2 free once copy gw-2 is done
                        tensor.wait_ge(cp_sem, gw - 1)
                    ps = psh[gw % 2]
                    for k in range(KT):
                        if first_wave:
                            w_gate(pos, piece_at_k, k)
                        for i in range(nw):
                            nt = nt0 + i
                            mm = tensor.matmul(
                                ps[:, i, 0:tw],
                                wbuf[
                                    :, r, k * N + 128 * nt : k * N + 128 * (nt + 1)
                                ],
                                xbuf[
                                    :,
                                    KT * so + k * cap + t0 : KT * so
                                    + k * cap
                                    + t0
                                    + tw,
                                ],
                                start=(k == 0),
                                stop=(k == KT - 1),
                                skip_group_check=True,
                            )
                            if k == KT - 1 and i == nw - 1:
                                mm.then_inc(mm_sem, 1)
                    first_wave = False
                    gw += 1

        @block.vector
        def _(vector):
            def copy(rb, cap, nt0, nw, t0, tw, src, gw):
                vector.wait_ge(mm_sem, gw + 1)
                dst = obuf[:, rb, nt0 * cap : (nt0 + nw) * cap].rearrange(
                    "p (nt t) -> p nt t", nt=nw
                )[:, :, t0 : t0 + tw]
                vector.tensor_scalar_mul(dst, src, OSCALE).then_inc(cp_sem, 1)

            # slot 0: three copies matching waves A/A/B
            cap = slot_caps[order[0]]
            copy(0, cap, 0, 4, 0, cap, ps0[:, 0:4, 0:cap], 0)
            copy(0, cap, 4, 4, 0, cap, ps1[:, 0:4, 0:cap], 1)
            copy(0, cap, 8, 3, 0, cap, ps0[:, 0:3, 0:cap], 2)
            gw = 3
            for pos in range(1, nslots):
                j = order[pos]
                cap = slot_caps[j]
                rb = pos % OB_RING
                first_wave = True
                for nt0, nw, t0, tw in waves_per_slot[pos]:
                    if first_wave and pos >= OB_RING:
                        # obuf ring slot free once out store pos-OB_RING done
                        vector.wait_ge(out_lanes[rb], 16 * (pos // OB_RING))
                    first_wave = False
                    copy(rb, cap, nt0, nw, t0, tw, psh[gw % 2][:, 0:nw, 0:tw], gw)
                    gw += 1

        @block.scalar
        def _(scalar):
            # slot 0's x in k-halves + slot 1's x, in parallel with the
            # slot-0/1 W stream
            cap0 = slot_caps[order[0]]
            for q in range(2):
                a = 8 * q * cap0
                b = 8 * (q + 1) * cap0
                scalar.dma_start(xbuf[:, a:b], xt_d[:, a:b]).then_inc(
                    x0_sems[q], 16
                )
            if nslots > 1:
                c0 = int(slot_offs[order[1]])
                c1 = int(slot_offs[order[1]] + slot_caps[order[1]])
                scalar.dma_start(
                    xbuf[:, KT * c0 : KT * c1], xt_d[:, KT * c0 : KT * c1]
                ).then_inc(x1_sem, 16)
            # per-slot out stores, trailing each slot's last wave copy
            for pos, j in enumerate(order):
                cap = slot_caps[j]
                so = int(slot_offs[j])
                rb = pos % OB_RING
                scalar.wait_ge(
                    cp_sem, cum_waves[-1] if DIAG_SER_OUT else cum_waves[pos]
                )
                scalar.dma_start(
                    out_d[:, NT * so : NT * (so + cap)],
                    obuf[:, rb, 0 : NT * cap],
                ).then_inc(out_lanes[rb], 16)
            # no final completion waits: the compiler epilogue's per-engine
            # DRAINs fence outstanding DMAs before the NEFF reports done

    # No end-of-run dma_reset/sem_clear: the start-of-run clear above
    # re-zeroes state. Restore allocator bases only.
    nc.psum_base, nc.psum_top = cleanup_psum
    nc.sbuf_base, nc.sbuf_top = cleanup_sbuf
    nc.compile()
    return nc


def _plan(bs):
    """Assign experts to (core, slot) and compute slot capacities."""
    order = np.argsort(-bs, kind="stable")  # experts sorted desc by size
    # slot j on core c handles expert order[8*j + c]
    assign = order.reshape(EPC, NCORES)
    caps = bs[assign].max(axis=1)
    caps = ((caps + 1) // 2) * 2  # keep token dim even
    return assign, caps.astype(np.int64)


def _prep_inputs(x, weight, bs, assign, caps):
    """Host-side shard/swizzle/quantize; cached (same arrays each call)."""
    key = (
        x.ctypes.data, weight.ctypes.data, x.shape, weight.shape,
        bs.tobytes(), tuple(int(c) for c in caps),
    )
    if key in _prep_cache:
        return _prep_cache[key]
    T_pad = int(caps.sum())
    offs = np.concatenate([[0], np.cumsum(bs)])
    slot_offs = np.concatenate([[0], np.cumsum(caps)])
    w3 = weight.reshape(E, M, N)

    xq = (x * XSCALE).astype(ml_dtypes.float8_e3m4)
    in_maps = []
    for c in range(NCORES):
        # per slot: (128, KT, cap) partition-major block of xT
        xt_core = np.zeros((128, KT * T_pad), dtype=ml_dtypes.float8_e3m4)
        w_core = np.empty((EPC, 128, KT * N), dtype=ml_dtypes.float8_e3m4)
        for j in range(EPC):
            e = int(assign[j, c])
            b = int(bs[e])
            blk = np.zeros((KT, 128, int(caps[j])), dtype=ml_dtypes.float8_e3m4)
            # xT rows (M=KT*128) for this slot's tokens
            blk[:, :, :b] = xq[offs[e] : offs[e] + b].T.reshape(KT, 128, b)
            xt_core[:, KT * slot_offs[j] : KT * slot_offs[j + 1]] = (
                blk.transpose(1, 0, 2).reshape(128, -1)
            )
            # W[e] (M,N) -> (KT,128,N) -> partition-major (128, KT*N)
            wq = (w3[e] * WSCALE).astype(ml_dtypes.float8_e3m4)
            w_core[j] = (
                wq.reshape(KT, 128, N).transpose(1, 0, 2).reshape(128, KT * N)
            )
        in_maps.append({"xt": xt_core, "w": w_core})
    _prep_cache.clear()
    _prep_cache[key] = in_maps
    return in_maps


def kernel(x: np.ndarray, weight: np.ndarray, batch_sizes: np.ndarray) -> np.ndarray:
    global LAST_EXEC_NS, LAST_RESULTS
    x = np.asarray(x)
    weight = np.asarray(weight)
    bs = np.asarray(batch_sizes).astype(np.int64)
    assert x.shape == (S, M) and weight.shape == (E * M, N)

    assign, caps = _plan(bs)
    key = tuple(caps.tolist())
    if key not in _program_cache:
        _program_cache[key] = _build_program(caps)
    nc = _program_cache[key]

    in_maps = _prep_inputs(x, weight, bs, assign, caps)

    trace = os.environ.get("BASS_KERNEL_TRACE", "1") != "0"
    try:
        res = run_bass_kernel_spmd(
            nc, in_maps, core_ids=list(range(NCORES)), trace=trace
        )
    except ModuleNotFoundError:
        # NTFF profiling hook unavailable in this image — run untraced.
        res = run_bass_kernel_spmd(
            nc, in_maps, core_ids=list(range(NCORES)), trace=False
        )
    LAST_RESULTS = res
    LAST_EXEC_NS = res.exec_time_ns

    offs = np.concatenate([[0], np.cumsum(bs)])
    slot_offs = np.concatenate([[0], np.cumsum(caps)])
    out = np.empty((S, N), dtype=np.float32)
    for c in range(NCORES):
        core_out = res.results[c]["out"]  # (128, NT*T_pad) bf16
        for j in range(EPC):
            e = int(assign[j, c])
            b = int(bs[e])
            cap = int(caps[j])
            so = int(slot_offs[j])
            blk = core_out[:, NT * so : NT * (so + cap)].reshape(128, NT, cap)
            out[offs[e] : offs[e] + b] = (
                blk.transpose(1, 0, 2).reshape(N, cap)[:, :b].T.astype(np.float32)
            )
    return out
